# revision 11
# baseline (speedup 1.0000x reference)
"""Multi-head causal attention (RoPE) for Trainium2, sharded over 8 NeuronCores.

Sharding: core c = 4*b + g handles batch b (of 2) and head-group g (4 of 16 heads).
Each core computes the qkv projection for its heads, RoPE, causal attention, and
a partial output projection (row-parallel over its heads' dims). The host sums
the 4 partials per batch and adds proj_b.

All on-chip compute is done in a transposed layout so no transposes are needed:
  - x is passed as xT (D, S); weights pre-transposed on the host.
  - q^T, k^T are produced as (head_dim, seq) tiles directly by the qkv matmul;
    V is produced in natural (seq, head_dim) layout by a second matmul.
  - scores are computed as S^T[k, q] with K=64 contraction, two heads per
    128-partition tile via row-tiled concurrent matmuls.
  - softmax runs over k (the partition dim); the row-sum l comes from an
    appended ones-column in V_aug during the PV matmul; normalization
    multiplies by a partition-broadcast 1/l.
  - o^T (dh, q) feeds the proj matmul as the stationary operand, producing the
    partial output in natural (s, j) layout for direct DMA out.

Schedule: the per-s-tile qkv work for s-tile st+1 is interleaved into the
attention kt-loop of q-tile st (attention for q-tile st only needs k-tiles
<= st), so PE has dense work while ScalarE runs the softmax exps; the output
projection for q-tile st follows its attention immediately.
"""
import numpy as np

B, S, D = 2, 2048, 1024
HEADS, HD = 16, 64
HALF = HD // 2
NCORES = 8
GROUPS = 4          # tensor-parallel head groups per batch
HPG = HEADS // GROUPS
O_QK = 2 * HPG * HD   # 512 rows: [q h0..h3 | k h0..h3]
O_V = HPG * HD        # 256
SCALE = 1.0 / np.sqrt(HD)

NST = S // 512        # 4 seq tiles of 512
NDC = D // 128        # 8 contraction chunks
NKT = S // 128        # 16 k tiles

_NC_CACHE = None

# schedule knobs (tuned against TimelineSim)
PSS_BUFS = 2
PSO_BUFS = 1
E_BUFS = 4
XT_BUFS = 10


def _build_nc():
    import concourse.bacc as bacc
    import concourse.mybir as mybir
    import concourse.tile as tile

    fp32 = mybir.dt.float32
    fp32r = mybir.dt.float32r
    Exp = mybir.ActivationFunctionType.Exp

    nc = bacc.Bacc(trn_type="TRN2", target_bir_lowering=False, debug=False)

    xT = nc.dram_tensor("xT", [D, S], fp32, kind="ExternalInput").ap()
    wqkT = nc.dram_tensor("wqkT", [D, O_QK], fp32, kind="ExternalInput").ap()
    wvT = nc.dram_tensor("wvT", [D, O_V], fp32, kind="ExternalInput").ap()
    bqk = nc.dram_tensor("bqk", [128, 4], fp32, kind="ExternalInput").ap()
    bv = nc.dram_tensor("bv", [64, 4], fp32, kind="ExternalInput").ap()
    cosF = nc.dram_tensor("cosF", [128, S], fp32, kind="ExternalInput").ap()
    sinF = nc.dram_tensor("sinF", [128, S], fp32, kind="ExternalInput").ap()
    pwT = nc.dram_tensor("pwT", [O_V, D], fp32, kind="ExternalInput").ap()
    ones = nc.dram_tensor("ones", [128, HPG], fp32, kind="ExternalInput").ap()
    outP = nc.dram_tensor("out_partial", [S, D], fp32, kind="ExternalOutput").ap()

    with tile.TileContext(nc) as tc:
        with tc.tile_pool(name="persist", bufs=1) as persist, \
             tc.tile_pool(name="xpool", bufs=XT_BUFS) as xpool, \
             tc.tile_pool(name="p1sb", bufs=3) as p1sb, \
             tc.tile_pool(name="p2sb", bufs=E_BUFS) as p2sb, \
             tc.tile_pool(name="nrm", bufs=2) as nrm, \
             tc.tile_pool(name="p3sb", bufs=3) as p3sb, \
             tc.tile_pool(name="ps1", bufs=2, space="PSUM") as ps1_pool, \
             tc.tile_pool(name="psS", bufs=PSS_BUFS, space="PSUM") as psS_pool, \
             tc.tile_pool(name="psOA", bufs=PSO_BUFS, space="PSUM") as psOA_pool, \
             tc.tile_pool(name="psOB", bufs=PSO_BUFS, space="PSUM") as psOB_pool:

            # --- persistent tiles -------------------------------------------
            wqk_t = [persist.tile([128, O_QK], fp32r, name=f"wqk{dc}", tag=f"wqk{dc}")
                     for dc in range(NDC)]
            wv_t = [persist.tile([128, O_V], fp32r, name=f"wv{dc}", tag=f"wv{dc}")
                    for dc in range(NDC)]
            pw_t = [persist.tile([128, D], fp32r, name=f"pw{dc}", tag=f"pw{dc}")
                    for dc in range(2)]
            cos_t = persist.tile([128, S], fp32, name="cos", tag="cos")
            sin_t = persist.tile([128, S], fp32, name="sin", tag="sin")
            bqk_t = persist.tile([128, 4], fp32, name="bqk", tag="bqk")
            bv_t = persist.tile([64, 4], fp32, name="bv", tag="bv")
            qk_t = [persist.tile([128, S], fp32r, name=f"qk{ot}", tag=f"qk{ot}")
                    for ot in range(4)]
            vaug_t = [persist.tile([128, HPG * (HD + 1)], fp32r, name=f"va{kt}", tag=f"va{kt}")
                      for kt in range(NKT)]
            outT_t = [persist.tile([128, S], fp32r, name=f"oT{t}", tag=f"oT{t}")
                      for t in range(2)]

            xt_tiles = [None] * NST

            def load_xt(st):
                s0 = st * 512
                xt = []
                for dc in range(NDC):
                    t = xpool.tile([128, 512], fp32r, name=f"xt{st}_{dc}", tag="xt")
                    nc.sync.dma_start(
                        out=t[:], in_=xT[dc * 128:(dc + 1) * 128, s0:s0 + 512].bitcast(fp32r))
                    xt.append(t)
                xt_tiles[st] = xt

            # --- load order: first s-tile + qk weights first ----------------
            load_xt(0)
            for dc in range(NDC):
                nc.sync.dma_start(out=wqk_t[dc][:],
                                  in_=wqkT[dc * 128:(dc + 1) * 128, :].bitcast(fp32r))
            load_xt(1)
            for dc in range(NDC):
                nc.sync.dma_start(out=wv_t[dc][:],
                                  in_=wvT[dc * 128:(dc + 1) * 128, :].bitcast(fp32r))
            nc.sync.dma_start(out=bqk_t[:], in_=bqk)
            nc.sync.dma_start(out=bv_t[:], in_=bv)
            nc.sync.dma_start(out=cos_t[:], in_=cosF)
            nc.sync.dma_start(out=sin_t[:], in_=sinF)
            for kt in range(NKT):
                nc.sync.dma_start(
                    out=vaug_t[kt].rearrange("p (h w) -> p h w", w=HD + 1)[:, :, HD:HD + 1],
                    in_=ones.bitcast(fp32r))
            for dc in range(2):
                nc.sync.dma_start(out=pw_t[dc][:],
                                  in_=pwT[dc * 128:(dc + 1) * 128, :].bitcast(fp32r))

            # --- phase-1 work, chunked for interleave -----------------------
            def emit_qk(st, ot):
                """One (o 128, s 512) q^T/k^T tile: matmul + bias + RoPE."""
                s0 = st * 512
                xt = xt_tiles[st]
                ps = ps1_pool.tile([128, 512], fp32, name=f"psA{st}_{ot}", tag="ps1")
                for dc in range(NDC):
                    nc.tensor.matmul(ps[:], wqk_t[dc][:, ot * 128:(ot + 1) * 128],
                                     xt[dc][:], start=(dc == 0), stop=(dc == NDC - 1))
                tb = p1sb.tile([128, 512], fp32, name=f"tb{st}_{ot}", tag="tb")
                nc.vector.tensor_scalar_add(tb[:], ps[:], bqk_t[:, ot:ot + 1])
                m2s = p1sb.tile([128, 512], fp32, name=f"m2s{st}_{ot}", tag="m2s")
                nc.vector.tensor_mul(m2s[:], tb[:], sin_t[:, s0:s0 + 512])
                m2 = p1sb.tile([128, 512], fp32, name=f"m2{st}_{ot}", tag="m2")
                for band in range(4):
                    dstp = band * 32
                    srcp = (band ^ 1) * 32
                    nc.vector.tensor_copy(m2[dstp:dstp + 32, :], m2s[srcp:srcp + 32, :])
                m1 = p1sb.tile([128, 512], fp32, name=f"m1{st}_{ot}", tag="m2s")
                nc.vector.tensor_mul(m1[:], tb[:], cos_t[:, s0:s0 + 512])
                nc.vector.tensor_add(qk_t[ot][:, s0:s0 + 512], m1[:], m2[:])

            def emit_v(st, ss):
                """One (s 128, o 256) natural-layout V tile into vaug."""
                xt = xt_tiles[st]
                ps = ps1_pool.tile([128, O_V], fp32, name=f"psB{st}_{ss}", tag="ps1")
                for dc in range(NDC):
                    nc.tensor.matmul(ps[:], xt[dc][:, ss * 128:(ss + 1) * 128],
                                     wv_t[dc][:], start=(dc == 0), stop=(dc == NDC - 1))
                vt = vaug_t[st * 4 + ss]
                nc.vector.tensor_copy(
                    vt.rearrange("p (h w) -> p h w", w=HD + 1)[:, :, 0:HD],
                    ps.rearrange("p (h w) -> p h w", w=HD))

            def phase1_chunks(st):
                ch = []
                for ot in range(4):
                    ch.append(lambda ot=ot: emit_qk(st, ot))
                for ss in range(4):
                    ch.append(lambda ss=ss: emit_v(st, ss))
                return ch

            # --- attention for one q-tile, software-pipelined ---------------
            def attention(qt, filler):
                """filler: list of phase-1 chunk closures to interleave."""
                q0 = qt * 512
                kmax = (q0 + 512) // 128
                n_steps = 2 * kmax
                fill_every = max(1, n_steps // max(1, len(filler))) if filler else 0
                step = 0

                def maybe_fill():
                    nonlocal step
                    step += 1
                    if filler and fill_every and step % fill_every == 0 and filler:
                        filler.pop(0)()

                for hp in range(2):
                    q_tile = qk_t[hp]
                    k_tile = qk_t[2 + hp]
                    hA, hB = 2 * hp, 2 * hp + 1
                    psO_A = psOA_pool.tile([HD + 1, 512], fp32, name=f"psOA{hp}_{qt}", tag="psOA")
                    psO_B = psOB_pool.tile([HD + 1, 512], fp32, name=f"psOB{hp}_{qt}", tag="psOB")
                    e_tiles = [None] * kmax

                    def emit_st(kt):
                        k0 = kt * 128
                        psS = psS_pool.tile([128, 1024], fp32, name=f"psS{hp}_{qt}_{kt}", tag="psS")
                        nc.tensor.matmul(psS[:, 0:512], k_tile[0:64, k0:k0 + 128],
                                         q_tile[0:64, q0:q0 + 512], start=True, stop=True,
                                         tile_position=(0, 0))
                        nc.tensor.matmul(psS[:, 512:1024], k_tile[64:128, k0:k0 + 128],
                                         q_tile[64:128, q0:q0 + 512], start=True, stop=True,
                                         tile_position=(64, 0))
                        e = p2sb.tile([128, 1024], fp32r, name=f"e{hp}_{qt}_{kt}", tag="e")
                        nc.scalar.activation(e[:], psS[:], Exp, scale=float(SCALE))
                        if k0 > q0 - 128:  # diagonal-crossing: zero where k0+p > q0+f
                            ear = e.rearrange("p (h q) -> p h q", q=512)
                            nc.gpsimd.affine_select(
                                out=ear, in_=ear,
                                compare_op=mybir.AluOpType.is_ge, fill=0.0,
                                base=-(k0 - q0), pattern=[[0, 2], [1, 512]],
                                channel_multiplier=-1)
                        e_tiles[kt] = e

                    def emit_pv(kt):
                        e = e_tiles[kt]
                        nc.tensor.matmul(psO_A[:], vaug_t[kt][:, hA * 65:hA * 65 + 65],
                                         e[:, 0:512], start=(kt == 0), stop=(kt == kmax - 1))
                        nc.tensor.matmul(psO_B[:], vaug_t[kt][:, hB * 65:hB * 65 + 65],
                                         e[:, 512:1024], start=(kt == 0), stop=(kt == kmax - 1))

                    emit_st(0)
                    for kt in range(kmax):
                        if kt + 1 < kmax:
                            emit_st(kt + 1)
                        emit_pv(kt)
                        maybe_fill()

                    for idx, psO in ((0, psO_A), (1, psO_B)):
                        h_local = 2 * hp + idx
                        rec = nrm.tile([1, 512], fp32, name=f"rec{hp}_{qt}_{idx}", tag="rec")
                        nc.vector.reciprocal(rec[0:1, :], psO[HD:HD + 1, :])
                        rbc = nrm.tile([64, 512], fp32, name=f"rbc{hp}_{qt}_{idx}", tag="rbc")
                        nc.gpsimd.partition_broadcast(rbc[:], rec[0:1, :])
                        nm = nrm.tile([64, 512], fp32, name=f"nm{hp}_{qt}_{idx}", tag="nm")
                        nc.vector.tensor_mul(nm[:], psO[0:64, :], rbc[:])
                        if idx == 0:
                            nc.vector.tensor_scalar_add(
                                outT_t[hp][0:64, q0:q0 + 512], nm[:],
                                bv_t[:, h_local:h_local + 1])
                        else:
                            tmpb = nrm.tile([64, 512], fp32, name=f"tb2{hp}_{qt}", tag="tb2")
                            nc.vector.tensor_scalar_add(tmpb[:], nm[:],
                                                        bv_t[:, h_local:h_local + 1])
                            nc.scalar.copy(outT_t[hp][64:128, q0:q0 + 512], tmpb[:])
                while filler:
                    filler.pop(0)()

            # --- output projection for one q-tile ---------------------------
            def proj(qt):
                for sst in range(4):
                    st128 = qt * 4 + sst
                    for jt in range(2):
                        ps = ps1_pool.tile([128, 512], fp32, name=f"psP{st128}_{jt}", tag="ps1")
                        for dc in range(2):
                            nc.tensor.matmul(ps[:], outT_t[dc][:, st128 * 128:(st128 + 1) * 128],
                                             pw_t[dc][:, jt * 512:(jt + 1) * 512],
                                             start=(dc == 0), stop=(dc == 1))
                        po = p3sb.tile([128, 512], fp32, name=f"po{st128}_{jt}", tag="po")
                        if (st128 + jt) % 2 == 0:
                            nc.scalar.copy(po[:], ps[:])
                        else:
                            nc.vector.tensor_copy(po[:], ps[:])
                        nc.sync.dma_start(
                            out=outP[st128 * 128:(st128 + 1) * 128, jt * 512:(jt + 1) * 512],
                            in_=po[:])

            # --- main schedule ----------------------------------------------
            for ch in phase1_chunks(0):
                ch()
            for qt in range(NST):
                if qt + 2 < NST:
                    load_xt(qt + 2)
                filler = phase1_chunks(qt + 1) if qt + 1 < NST else []
                attention(qt, filler)
                proj(qt)

    nc.compile()
    return nc


def _get_nc():
    global _NC_CACHE
    if _NC_CACHE is None:
        _NC_CACHE = _build_nc()
    return _NC_CACHE


def _prep_in_maps(x, qkv_w, qkv_b, proj_w, proj_b):
    x = np.asarray(x, dtype=np.float32)
    qkv_w = np.asarray(qkv_w, dtype=np.float32)
    qkv_b = np.asarray(qkv_b, dtype=np.float32)
    proj_w = np.asarray(proj_w, dtype=np.float32)

    # RoPE tables (match reference fp32 math)
    freqs = (1.0 / (10000.0 ** (np.arange(HALF, dtype=np.float32) / HALF))).astype(np.float32)
    pos = np.arange(S, dtype=np.float32)
    ang = pos[:, None] * freqs[None, :]            # (S, 32)
    cos_m = np.cos(ang).astype(np.float32)
    sin_m = np.sin(ang).astype(np.float32)
    p = np.arange(128)
    cosF = np.ascontiguousarray(cos_m[:, p % HALF].T)            # (128, S)
    sign = np.where((p % HD) < HALF, 1.0, -1.0).astype(np.float32)
    sinF = np.ascontiguousarray((sin_m[:, p % HALF] * sign[None, :]).T)

    in_maps = []
    for c in range(NCORES):
        b, g = divmod(c, GROUPS)
        hs = np.arange(HPG) + HPG * g
        q_rows = np.concatenate([np.arange(HD * h, HD * h + HD) for h in hs])
        qk_rows = np.concatenate([q_rows, q_rows + D])
        v_rows = q_rows + 2 * D
        in_maps.append({
            "xT": np.ascontiguousarray(x[b].T),
            "wqkT": np.ascontiguousarray(qkv_w[qk_rows, :].T),
            "wvT": np.ascontiguousarray(qkv_w[v_rows, :].T),
            "bqk": np.ascontiguousarray(qkv_b[qk_rows].reshape(4, 128).T),
            "bv": np.ascontiguousarray(qkv_b[v_rows].reshape(4, HD).T),
            "cosF": cosF,
            "sinF": sinF,
            "pwT": np.ascontiguousarray(proj_w[:, q_rows].T),
            "ones": np.ones((128, HPG), dtype=np.float32),
        })
    return in_maps


def _run(x, qkv_w, qkv_b, proj_w, proj_b, trace=False):
    from concourse import bass_utils
    nc = _get_nc()
    in_maps = _prep_in_maps(x, qkv_w, qkv_b, proj_w, proj_b)
    res = bass_utils.run_bass_kernel_spmd(nc, in_maps, core_ids=list(range(NCORES)),
                                          trace=trace)
    proj_b = np.asarray(proj_b, dtype=np.float32)
    out = np.empty((B, S, D), dtype=np.float32)
    for b in range(B):
        acc = np.zeros((S, D), dtype=np.float64)
        for g in range(GROUPS):
            acc += res.results[b * GROUPS + g]["out_partial"]
        out[b] = (acc + proj_b[None, :]).astype(np.float32)
    return out, res


def kernel(x, qkv_w, qkv_b, proj_w, proj_b):
    out, _ = _run(x, qkv_w, qkv_b, proj_w, proj_b, trace=False)
    return out
